# revision 16
# baseline (speedup 1.0000x reference)
"""Chamfer distance (nn_ChamferDistance) Trainium2 Bass kernel.

Computes, for xyz1/xyz2 of shape (4, 8192, 3) fp32:
    dist[n, m] = |p_n|^2 + |q_m|^2 - 2 p_n.q_m   (per batch)
    dist1 = min over m, dist2 = min over n
Returns (dist1, dist2), each (4, 8192) fp32 — same as the reference.

Strategy (single-pass, negated):
  - The pairwise-distance matrix is produced directly by the TensorEngine via
    an augmented inner product: u_a . v_b = sq(P)[a] + sq(Q)[b] - 2 P_a.Q_b.
    All factors are split into 3 bf16 planes (hi/lo/lolo) so every product the
    PE forms is exact in fp32; dropped cross terms are ~2^-26 relative.  The
    L-side planes are negated on the host so the device computes -dist and
    every reduction becomes a MAX.  Host negates the outputs back.
  - Sharding: 8 cores = 4 batches x 2 halves of N.  Each core computes its
    4096 x 8192 block of -dist ONCE:
      * dist1 rows come from a per-tile free-axis max,
      * dist2 comes from an elementwise column-accumulator max across the 32
        row-tiles; the partition-axis reduction of the accumulator and the
        combination of the two N-halves happen on the host (the [128, 8192]
        fp16 accumulator is DMA'd out in chunks; this is ~free on-device,
        whereas the gpsimd partition_all_reduce it replaces ran 27us and
        contended with the DVE for the shared SBUF port).
  - Per 128-row tile, 16 matmuls fill 4 PSUM groups of [128, 2048].  The
    ScalarEngine drains each group to SBUF with an fp32->fp16 downcast
    (fp16 keeps min errors ~2^-11; tolerance is 2e-2).  The VectorEngine then
    consumes each staged tile with 2x-mode tensor_tensor ops only - measured:
    TT fp16 runs at 2 elem/cyc while every reduce-shaped op (tensor_reduce,
    Max8, Pool, tensor_scalar+accum) runs at 1 elem/cyc and
    tensor_tensor_reduce crashes the exec unit:
      * row-max: TT-max fold tree 8192->4096->...->256 plus one 1x
        tensor_reduce on the last 256 (~4.6us vs 8.7us flat reduce).
        Four row-tiles share one scr buffer so levels L2..L5 and the final
        reduce run once per quad over [128, 4, n] strided APs (the 2x-1p
        mode check only looks at the innermost dim), cutting instruction
        overhead.
      * column accumulator: one TT max per tile (4.3us).
  - Pipeline shaping (the kernel is DVE-bound; end ~= DVE_start + DVE_busy +
    ramp gaps + tail; note ACT at ~7.7us/tile is only slightly below the
    DVE's ~8.8us/tile, so the tree work must stay interleaved at quad
    granularity - batching it per-octet makes the DVE overrun the ACT
    mid-stream and stall ~2.7us per octet, measured):
      * input DMAs are split (first lhsT block + first rhs 512 first) so
        the first matmul starts as early as the DMA infra allows (~11us);
      * PSUM groups run in order g0,g2,g1,g3; tiles 0-3 run per-group
        colacc updates and split L1 halves so the DVE follows the PE's slow
        (mid p-state, 464ns/matmul) ramp cadence instead of waiting for
        whole tiles;
      * steady-state L1 folds are merged across tile pairs ([128, 2, 4096]
        APs over a shared stage buffer) to cut per-op overhead;
      * the row tree folds to width 128 before the 1x-rate tensor_reduce
        (a 2x TT fold is cheaper than widening the reduce, down to the
        ~120ns/op overhead crossover);
      * d1 is DMA'd out per quad so only a tiny DMA trails the last reduce;
      * in the LAST quad the L1 folds are deferred until after the final
        colacc update so the colacc DMA-out overlaps the deferred tree work.
"""

import numpy as np
import ml_dtypes

import concourse.bacc as bacc
import concourse.tile as tile
import concourse.mybir as mybir
from concourse import bass_utils

B = 4
N = 8192
M = 8192
NCORES = 8
NSH = N // 2          # rows per core
NT = NSH // 128       # 32 row tiles per core
K = 24                # augmented contraction rows
GF = 2048             # PSUM drain group size (4 banks; 2 groups in flight)

BF16 = mybir.dt.bfloat16
F16 = mybir.dt.float16
F32 = mybir.dt.float32
MAX = mybir.AluOpType.max
X = mybir.AxisListType.X

# Group order: L1's first half reads groups {0, 2}, second half {1, 3}.
# Producing g0, g2 first lets tile 0's split DVE ops start after 2 drains.
GORDER = (0, 2, 1, 3)


def build_body(tc, lhsT, rhs, d1t, d2t, repeat=1):
    """Emit the kernel body into TileContext `tc`.

    lhsT: [K, NT*128] bf16 AP  (negated augmented rows of this core's N-half)
    rhs:  [K, M]      bf16 AP  (augmented rows of all of xyz2[b])
    d1t:  [NT/4, 128, 4] f32 AP out (quad-major; [q, r, c] -> -dist1 of
          point (4q+c)*128 + r)
    d2t:  [128, M] f16 AP out (-dist column accumulator; host max-reduces
          over the partition axis)
    """
    nc = tc.nc
    nj = GF // 512
    with (
        tc.tile_pool(name="inp", bufs=1) as inp_pool,
        tc.tile_pool(name="acc", bufs=1) as acc_pool,
        tc.tile_pool(name="stage", bufs=3) as stage_pool,
        tc.tile_pool(name="scr", bufs=2) as scr_pool,
        tc.tile_pool(name="cacc", bufs=1) as cacc_pool,
        tc.tile_pool(name="psum", bufs=2, space="PSUM") as psum_pool,
    ):
        ls = inp_pool.tile([K, NT * 128], BF16, tag="ls")
        rs = inp_pool.tile([K, M], BF16, tag="rs")
        # First matmul only needs rs[:, 0:512] + ls[:, 0:128]; issue those
        # two small DMAs first so the PE starts as soon as possible.
        nc.sync.dma_start(rs[:, 0:512], rhs[:, 0:512])
        nc.sync.dma_start(ls[:, 0:128], lhsT[:, 0:128])
        nc.sync.dma_start(ls[:, 128:NT * 128], lhsT[:, 128:NT * 128])
        nc.sync.dma_start(rs[:, 512:GF], rhs[:, 512:GF])
        for g in GORDER[1:]:
            nc.sync.dma_start(rs[:, g * GF:(g + 1) * GF],
                              rhs[:, g * GF:(g + 1) * GF])

        d1 = acc_pool.tile([128, NT], F32, tag="d1")
        colacc = cacc_pool.tile([128, M], F16, tag="cacc")

        QT = 4   # row-tiles per scr group
        for _ in range(repeat):
            for ip in range(NT // QT):
                first = ip == 0
                last = ip == NT // QT - 1
                scr = scr_pool.tile([128, QT, 4096], F16, tag="scr")
                deferred = []
                for kp in range(QT // 2):
                    # Two row-tiles share one stage buffer so their L1 folds
                    # merge into a single [128, 2, 4096] op (saves the per-op
                    # init + pipe overhead 16x per core).
                    ramp = ip < 2
                    stp = stage_pool.tile([128, 2, M], F16, tag="st")
                    for u in range(2):
                        k = 2 * kp + u
                        i = QT * ip + k
                        st = stp[:, u, :]
                        for g in GORDER:
                            ps = psum_pool.tile([128, GF], F32, tag="ps")
                            for j in range(nj):
                                nc.tensor.matmul(
                                    ps[:, j * 512:(j + 1) * 512],
                                    ls[:, i * 128:(i + 1) * 128],
                                    rs[:, g * GF + j * 512: g * GF + (j + 1) * 512],
                                    start=True,
                                    stop=True,
                                )
                            nc.scalar.copy(st[:, g * GF:(g + 1) * GF], ps[:])
                        if i == 0:
                            # Fast start: per-group colacc init (4x-mode
                            # copies) + split L1 halves, ready after 2 drains.
                            nc.vector.tensor_copy(colacc[:, 0:GF], st[:, 0:GF])
                            nc.vector.tensor_copy(colacc[:, 2 * GF:3 * GF],
                                                  st[:, 2 * GF:3 * GF])
                            nc.vector.tensor_tensor(scr[:, 0, 0:GF],
                                                    st[:, 0:GF],
                                                    st[:, 2 * GF:3 * GF],
                                                    op=MAX)
                            nc.vector.tensor_copy(colacc[:, GF:2 * GF],
                                                  st[:, GF:2 * GF])
                            nc.vector.tensor_copy(colacc[:, 3 * GF:4 * GF],
                                                  st[:, 3 * GF:4 * GF])
                            nc.vector.tensor_tensor(scr[:, 0, GF:2 * GF],
                                                    st[:, GF:2 * GF],
                                                    st[:, 3 * GF:4 * GF],
                                                    op=MAX)
                            continue
                        if ramp:
                            # Ramp (tiles 1-3): per-group colacc updates and
                            # split L1 halves keep the DVE busy at the PE's
                            # (slower) early cadence instead of waiting for
                            # all four drains.
                            for g, half in ((0, 0), (2, 0), (1, 1), (3, 1)):
                                nc.vector.tensor_tensor(
                                    colacc[:, g * GF:(g + 1) * GF],
                                    st[:, g * GF:(g + 1) * GF],
                                    colacc[:, g * GF:(g + 1) * GF], op=MAX)
                                if g >= 2:
                                    nc.vector.tensor_tensor(
                                        scr[:, k, half * GF:(half + 1) * GF],
                                        st[:, half * GF:(half + 1) * GF],
                                        st[:, (half + 2) * GF:(half + 3) * GF],
                                        op=MAX)
                            continue
                        nc.vector.tensor_tensor(colacc[:], st[:], colacc[:],
                                                op=MAX)
                    if ramp:
                        continue
                    if last:
                        deferred.append((kp, stp))
                    else:
                        nc.vector.tensor_tensor(scr[:, 2 * kp:2 * kp + 2, :4096],
                                                stp[:, :, 0:4096],
                                                stp[:, :, 4096:8192], op=MAX)
                # In the LAST quad the L1 folds run after the final colacc
                # update so the colacc DMA-out overlaps the remaining tree.
                for kp, stp in deferred:
                    nc.vector.tensor_tensor(scr[:, 2 * kp:2 * kp + 2, :4096],
                                            stp[:, :, 0:4096],
                                            stp[:, :, 4096:8192], op=MAX)
                # Folds run IN PLACE: out[c] = max(in[c], in[c+w]) streams
                # reads ahead of the 8-stage-delayed writes.  The extra L128
                # level is cheaper at 2x than widening the 1x tensor_reduce.
                for w in (2048, 1024, 512, 256, 128):
                    nc.vector.tensor_tensor(scr[:, :, :w], scr[:, :, :w],
                                            scr[:, :, w:2 * w], op=MAX)
                nc.vector.tensor_reduce(d1[:, QT * ip:QT * (ip + 1)],
                                        scr[:, :, :128], axis=X, op=MAX)
                nc.sync.dma_start(d1t[ip], d1[:, QT * ip:QT * (ip + 1)])

        # Ship the raw column accumulator; host does the partition reduce.
        # Chunked so the transfer spreads over several DMA queues and hides
        # behind the deferred last-quad tree work.
        NCH = 8
        w = M // NCH
        for c in range(NCH):
            nc.sync.dma_start(d2t[:, c * w:(c + 1) * w],
                              colacc[:, c * w:(c + 1) * w])


def build_kernel(nc, repeat=1):
    lhsT = nc.dram_tensor("lhsT", [K, NT * 128], BF16, kind="ExternalInput")
    rhs = nc.dram_tensor("rhs", [K, M], BF16, kind="ExternalInput")
    # Quad-major layout: each per-quad DMA is one contiguous 2KB burst
    # instead of 128 scattered 16B rows (matters for the last, tail DMA).
    d1t = nc.dram_tensor("d1t", [NT // 4, 128, 4], F32,
                         kind="ExternalOutput")
    d2t = nc.dram_tensor("d2t", [128, M], F16, kind="ExternalOutput")
    with tile.TileContext(nc) as tc:
        build_body(tc, lhsT.ap(), rhs.ap(), d1t.ap(), d2t.ap(), repeat)
    return nc


def _split3(v):
    """v (fp32) -> three bf16 planes (as fp32) with v ~= h + l + ll."""
    bf = ml_dtypes.bfloat16
    h = v.astype(bf).astype(np.float32)
    l = (v - h).astype(bf).astype(np.float32)
    ll = (v - h - l).astype(bf).astype(np.float32)
    return h, l, ll


def _build_aug(x1, x2):
    """x1 [n,3], x2 [m,3] fp32 -> (L [24,n] bf16, R [24,m] bf16) with
    (L.T @ R)[a,b] ~= -(|x1_a|^2 + |x2_b|^2 - 2 x1_a.x2_b)  (negated)."""
    n = x1.shape[0]
    m = x2.shape[0]
    sq1 = (x1 * x1).sum(-1)
    sq2 = (x2 * x2).sum(-1)
    a = -2.0 * x1
    y = x2
    s1h, s1l, s1ll = _split3(sq1)
    s2h, s2l, s2ll = _split3(sq2)
    ah, al, all_ = _split3(a)
    yh, yl, yll = _split3(y)
    ones_n = np.ones(n, np.float32)
    ones_m = np.ones(m, np.float32)
    Ls = [s1h, s1l, s1ll, ones_n, ones_n, ones_n]
    Rs = [ones_m, ones_m, ones_m, s2h, s2l, s2ll]
    for c in range(3):
        for (L, R) in ((ah, yh), (ah, yl), (ah, yll), (al, yh), (al, yl), (all_, yh)):
            Ls.append(L[:, c])
            Rs.append(R[:, c])
    bf = ml_dtypes.bfloat16
    Lm = np.ascontiguousarray(-np.stack(Ls)).astype(bf)   # negated
    Rm = np.ascontiguousarray(np.stack(Rs)).astype(bf)
    return Lm, Rm


def _make_in_maps(xyz1, xyz2):
    in_maps = []
    for c in range(NCORES):
        b, h = divmod(c, 2)
        L, R = _build_aug(xyz1[b, h * NSH:(h + 1) * NSH], xyz2[b])
        in_maps.append({"lhsT": L, "rhs": R})
    return in_maps


_CACHE = {}


def _get_compiled(repeat=1):
    key = ("nc", repeat)
    if key not in _CACHE:
        nc = bacc.Bacc("TRN2", target_bir_lowering=False, debug=False,
                       num_devices=NCORES)
        build_kernel(nc, repeat=repeat)
        nc.compile()
        _CACHE[key] = nc
    return _CACHE[key]


def _gather(results):
    d1 = np.empty((B, N), np.float32)
    d2 = np.empty((B, M), np.float32)
    for b in range(B):
        r0 = results[2 * b]
        r1 = results[2 * b + 1]
        d1[b, :NSH] = -r0["d1t"].transpose(1, 0, 2).reshape(128, NT).T.reshape(-1)
        d1[b, NSH:] = -r1["d1t"].transpose(1, 0, 2).reshape(128, NT).T.reshape(-1)
        m0 = r0["d2t"].astype(np.float32).max(axis=0)
        m1 = r1["d2t"].astype(np.float32).max(axis=0)
        d2[b] = -np.maximum(m0, m1)
    return d1, d2


def kernel(xyz1, xyz2):
    xyz1 = np.asarray(xyz1, dtype=np.float32)
    xyz2 = np.asarray(xyz2, dtype=np.float32)
    in_maps = _make_in_maps(xyz1, xyz2)
    nc = _get_compiled()
    res = bass_utils.run_bass_kernel_spmd(nc, in_maps, core_ids=list(range(NCORES)))
    return _gather(res.results)


# revision 17
# speedup vs baseline: 1.0011x; 1.0011x over previous
"""Chamfer distance (nn_ChamferDistance) Trainium2 Bass kernel.

Computes, for xyz1/xyz2 of shape (4, 8192, 3) fp32:
    dist[n, m] = |p_n|^2 + |q_m|^2 - 2 p_n.q_m   (per batch)
    dist1 = min over m, dist2 = min over n
Returns (dist1, dist2), each (4, 8192) fp32 — same as the reference.

Strategy (single-pass, negated):
  - The pairwise-distance matrix is produced directly by the TensorEngine via
    an augmented inner product: u_a . v_b = sq(P)[a] + sq(Q)[b] - 2 P_a.Q_b.
    All factors are split into 3 bf16 planes (hi/lo/lolo) so every product the
    PE forms is exact in fp32; dropped cross terms are ~2^-26 relative.  The
    L-side planes are negated on the host so the device computes -dist and
    every reduction becomes a MAX.  Host negates the outputs back.
  - Sharding: 8 cores = 4 batches x 2 halves of N.  Each core computes its
    4096 x 8192 block of -dist ONCE:
      * dist1 rows come from a per-tile free-axis max,
      * dist2 comes from an elementwise column-accumulator max across the 32
        row-tiles; the partition-axis reduction of the accumulator and the
        combination of the two N-halves happen on the host (the [128, 8192]
        fp16 accumulator is DMA'd out in chunks; this is ~free on-device,
        whereas the gpsimd partition_all_reduce it replaces ran 27us and
        contended with the DVE for the shared SBUF port).
  - Per 128-row tile, 16 matmuls fill 4 PSUM groups of [128, 2048].  The
    ScalarEngine drains each group to SBUF with an fp32->fp16 downcast
    (fp16 keeps min errors ~2^-11; tolerance is 2e-2).  The VectorEngine then
    consumes each staged tile with 2x-mode tensor_tensor ops only - measured:
    TT fp16 runs at 2 elem/cyc while every reduce-shaped op (tensor_reduce,
    Max8, Pool, tensor_scalar+accum) runs at 1 elem/cyc and
    tensor_tensor_reduce crashes the exec unit:
      * row-max: TT-max fold tree 8192->4096->...->256 plus one 1x
        tensor_reduce on the last 256 (~4.6us vs 8.7us flat reduce).
        Four row-tiles share one scr buffer so levels L2..L5 and the final
        reduce run once per quad over [128, 4, n] strided APs (the 2x-1p
        mode check only looks at the innermost dim), cutting instruction
        overhead.
      * column accumulator: one TT max per tile (4.3us).
  - Pipeline shaping (the kernel is DVE-bound; end ~= DVE_start + DVE_busy +
    ramp gaps + tail; note ACT at ~7.7us/tile is only slightly below the
    DVE's ~8.8us/tile, so the tree work must stay interleaved at quad
    granularity - batching it per-octet makes the DVE overrun the ACT
    mid-stream and stall ~2.7us per octet, measured):
      * input DMAs are split (first lhsT block + first rhs 512 first) so
        the first matmul starts as early as the DMA infra allows (~11us);
      * PSUM groups run in order g0,g2,g1,g3; tiles 0-3 run per-group
        colacc updates and split L1 halves so the DVE follows the PE's slow
        (mid p-state, 464ns/matmul) ramp cadence instead of waiting for
        whole tiles;
      * steady-state L1 folds are merged across tile pairs ([128, 2, 4096]
        APs over a shared stage buffer) to cut per-op overhead;
      * the row tree folds to width 128 before the 1x-rate tensor_reduce
        (a 2x TT fold is cheaper than widening the reduce, down to the
        ~120ns/op overhead crossover);
      * d1 is DMA'd out per quad in a quad-major DRAM layout (one
        contiguous 2KB burst per quad instead of 128 scattered 16B rows),
        so only a ~0.6us DMA trails the last reduce; the residual ~10us
        tail is runtime-fixed (end barrier + notification-queue DMAs);
      * in the LAST quad the L1 folds are deferred until after the final
        colacc update so the colacc DMA-out overlaps the deferred tree work.
"""

import numpy as np
import ml_dtypes

import concourse.bacc as bacc
import concourse.tile as tile
import concourse.mybir as mybir
from concourse import bass_utils

B = 4
N = 8192
M = 8192
NCORES = 8
NSH = N // 2          # rows per core
NT = NSH // 128       # 32 row tiles per core
K = 24                # augmented contraction rows
GF = 2048             # PSUM drain group size (4 banks; 2 groups in flight)

BF16 = mybir.dt.bfloat16
F16 = mybir.dt.float16
F32 = mybir.dt.float32
MAX = mybir.AluOpType.max
X = mybir.AxisListType.X

# Group order: L1's first half reads groups {0, 2}, second half {1, 3}.
# Producing g0, g2 first lets tile 0's split DVE ops start after 2 drains.
GORDER = (0, 2, 1, 3)


def build_body(tc, lhsT, rhs, d1t, d2t, repeat=1):
    """Emit the kernel body into TileContext `tc`.

    lhsT: [K, NT*128] bf16 AP  (negated augmented rows of this core's N-half)
    rhs:  [K, M]      bf16 AP  (augmented rows of all of xyz2[b])
    d1t:  [NT/4, 128, 4] f32 AP out (quad-major; [q, r, c] -> -dist1 of
          point (4q+c)*128 + r)
    d2t:  [128, M] f16 AP out (-dist column accumulator; host max-reduces
          over the partition axis)
    """
    nc = tc.nc
    nj = GF // 512
    with (
        tc.tile_pool(name="inp", bufs=1) as inp_pool,
        tc.tile_pool(name="acc", bufs=1) as acc_pool,
        tc.tile_pool(name="stage", bufs=3) as stage_pool,
        tc.tile_pool(name="scr", bufs=2) as scr_pool,
        tc.tile_pool(name="cacc", bufs=1) as cacc_pool,
        tc.tile_pool(name="psum", bufs=2, space="PSUM") as psum_pool,
    ):
        ls = inp_pool.tile([K, NT * 128], BF16, tag="ls")
        rs = inp_pool.tile([K, M], BF16, tag="rs")
        # First matmul only needs rs[:, 0:512] + ls[:, 0:128]; issue those
        # two small DMAs first so the PE starts as soon as possible.
        nc.sync.dma_start(rs[:, 0:512], rhs[:, 0:512])
        nc.sync.dma_start(ls[:, 0:128], lhsT[:, 0:128])
        nc.sync.dma_start(ls[:, 128:NT * 128], lhsT[:, 128:NT * 128])
        nc.sync.dma_start(rs[:, 512:GF], rhs[:, 512:GF])
        for g in GORDER[1:]:
            nc.sync.dma_start(rs[:, g * GF:(g + 1) * GF],
                              rhs[:, g * GF:(g + 1) * GF])

        d1 = acc_pool.tile([128, NT], F32, tag="d1")
        colacc = cacc_pool.tile([128, M], F16, tag="cacc")

        QT = 4   # row-tiles per scr group
        for _ in range(repeat):
            for ip in range(NT // QT):
                first = ip == 0
                last = ip == NT // QT - 1
                scr = scr_pool.tile([128, QT, 4096], F16, tag="scr")
                deferred = []
                for kp in range(QT // 2):
                    # Two row-tiles share one stage buffer so their L1 folds
                    # merge into a single [128, 2, 4096] op (saves the per-op
                    # init + pipe overhead 16x per core).
                    ramp = ip < 2
                    stp = stage_pool.tile([128, 2, M], F16, tag="st")
                    for u in range(2):
                        k = 2 * kp + u
                        i = QT * ip + k
                        st = stp[:, u, :]
                        for g in GORDER:
                            ps = psum_pool.tile([128, GF], F32, tag="ps")
                            for j in range(nj):
                                nc.tensor.matmul(
                                    ps[:, j * 512:(j + 1) * 512],
                                    ls[:, i * 128:(i + 1) * 128],
                                    rs[:, g * GF + j * 512: g * GF + (j + 1) * 512],
                                    start=True,
                                    stop=True,
                                )
                            nc.scalar.copy(st[:, g * GF:(g + 1) * GF], ps[:])
                        if i == 0:
                            # Fast start: per-group colacc init (4x-mode
                            # copies) + split L1 halves, ready after 2 drains.
                            nc.vector.tensor_copy(colacc[:, 0:GF], st[:, 0:GF])
                            nc.vector.tensor_copy(colacc[:, 2 * GF:3 * GF],
                                                  st[:, 2 * GF:3 * GF])
                            nc.vector.tensor_tensor(scr[:, 0, 0:GF],
                                                    st[:, 0:GF],
                                                    st[:, 2 * GF:3 * GF],
                                                    op=MAX)
                            nc.vector.tensor_copy(colacc[:, GF:2 * GF],
                                                  st[:, GF:2 * GF])
                            nc.vector.tensor_copy(colacc[:, 3 * GF:4 * GF],
                                                  st[:, 3 * GF:4 * GF])
                            nc.vector.tensor_tensor(scr[:, 0, GF:2 * GF],
                                                    st[:, GF:2 * GF],
                                                    st[:, 3 * GF:4 * GF],
                                                    op=MAX)
                            continue
                        if ramp:
                            # Ramp (tiles 1-3): per-group colacc updates and
                            # split L1 halves keep the DVE busy at the PE's
                            # (slower) early cadence instead of waiting for
                            # all four drains.
                            for g, half in ((0, 0), (2, 0), (1, 1), (3, 1)):
                                nc.vector.tensor_tensor(
                                    colacc[:, g * GF:(g + 1) * GF],
                                    st[:, g * GF:(g + 1) * GF],
                                    colacc[:, g * GF:(g + 1) * GF], op=MAX)
                                if g >= 2:
                                    nc.vector.tensor_tensor(
                                        scr[:, k, half * GF:(half + 1) * GF],
                                        st[:, half * GF:(half + 1) * GF],
                                        st[:, (half + 2) * GF:(half + 3) * GF],
                                        op=MAX)
                            continue
                        nc.vector.tensor_tensor(colacc[:], st[:], colacc[:],
                                                op=MAX)
                    if ramp:
                        continue
                    if last:
                        deferred.append((kp, stp))
                    else:
                        nc.vector.tensor_tensor(scr[:, 2 * kp:2 * kp + 2, :4096],
                                                stp[:, :, 0:4096],
                                                stp[:, :, 4096:8192], op=MAX)
                # In the LAST quad the L1 folds run after the final colacc
                # update so the colacc DMA-out overlaps the remaining tree.
                for kp, stp in deferred:
                    nc.vector.tensor_tensor(scr[:, 2 * kp:2 * kp + 2, :4096],
                                            stp[:, :, 0:4096],
                                            stp[:, :, 4096:8192], op=MAX)
                # Folds run IN PLACE: out[c] = max(in[c], in[c+w]) streams
                # reads ahead of the 8-stage-delayed writes.  The extra L128
                # level is cheaper at 2x than widening the 1x tensor_reduce.
                for w in (2048, 1024, 512, 256, 128):
                    nc.vector.tensor_tensor(scr[:, :, :w], scr[:, :, :w],
                                            scr[:, :, w:2 * w], op=MAX)
                nc.vector.tensor_reduce(d1[:, QT * ip:QT * (ip + 1)],
                                        scr[:, :, :128], axis=X, op=MAX)
                nc.sync.dma_start(d1t[ip], d1[:, QT * ip:QT * (ip + 1)])

        # Ship the raw column accumulator; host does the partition reduce.
        # Chunked so the transfer spreads over several DMA queues and hides
        # behind the deferred last-quad tree work.
        NCH = 8
        w = M // NCH
        for c in range(NCH):
            nc.sync.dma_start(d2t[:, c * w:(c + 1) * w],
                              colacc[:, c * w:(c + 1) * w])


def build_kernel(nc, repeat=1):
    lhsT = nc.dram_tensor("lhsT", [K, NT * 128], BF16, kind="ExternalInput")
    rhs = nc.dram_tensor("rhs", [K, M], BF16, kind="ExternalInput")
    # Quad-major layout: each per-quad DMA is one contiguous 2KB burst
    # instead of 128 scattered 16B rows (matters for the last, tail DMA).
    d1t = nc.dram_tensor("d1t", [NT // 4, 128, 4], F32,
                         kind="ExternalOutput")
    d2t = nc.dram_tensor("d2t", [128, M], F16, kind="ExternalOutput")
    with tile.TileContext(nc) as tc:
        build_body(tc, lhsT.ap(), rhs.ap(), d1t.ap(), d2t.ap(), repeat)
    return nc


def _split3(v):
    """v (fp32) -> three bf16 planes (as fp32) with v ~= h + l + ll."""
    bf = ml_dtypes.bfloat16
    h = v.astype(bf).astype(np.float32)
    l = (v - h).astype(bf).astype(np.float32)
    ll = (v - h - l).astype(bf).astype(np.float32)
    return h, l, ll


def _build_aug(x1, x2):
    """x1 [n,3], x2 [m,3] fp32 -> (L [24,n] bf16, R [24,m] bf16) with
    (L.T @ R)[a,b] ~= -(|x1_a|^2 + |x2_b|^2 - 2 x1_a.x2_b)  (negated)."""
    n = x1.shape[0]
    m = x2.shape[0]
    sq1 = (x1 * x1).sum(-1)
    sq2 = (x2 * x2).sum(-1)
    a = -2.0 * x1
    y = x2
    s1h, s1l, s1ll = _split3(sq1)
    s2h, s2l, s2ll = _split3(sq2)
    ah, al, all_ = _split3(a)
    yh, yl, yll = _split3(y)
    ones_n = np.ones(n, np.float32)
    ones_m = np.ones(m, np.float32)
    Ls = [s1h, s1l, s1ll, ones_n, ones_n, ones_n]
    Rs = [ones_m, ones_m, ones_m, s2h, s2l, s2ll]
    for c in range(3):
        for (L, R) in ((ah, yh), (ah, yl), (ah, yll), (al, yh), (al, yl), (all_, yh)):
            Ls.append(L[:, c])
            Rs.append(R[:, c])
    bf = ml_dtypes.bfloat16
    Lm = np.ascontiguousarray(-np.stack(Ls)).astype(bf)   # negated
    Rm = np.ascontiguousarray(np.stack(Rs)).astype(bf)
    return Lm, Rm


def _make_in_maps(xyz1, xyz2):
    in_maps = []
    for c in range(NCORES):
        b, h = divmod(c, 2)
        L, R = _build_aug(xyz1[b, h * NSH:(h + 1) * NSH], xyz2[b])
        in_maps.append({"lhsT": L, "rhs": R})
    return in_maps


_CACHE = {}


def _get_compiled(repeat=1):
    key = ("nc", repeat)
    if key not in _CACHE:
        nc = bacc.Bacc("TRN2", target_bir_lowering=False, debug=False,
                       num_devices=NCORES)
        build_kernel(nc, repeat=repeat)
        nc.compile()
        _CACHE[key] = nc
    return _CACHE[key]


def _gather(results):
    d1 = np.empty((B, N), np.float32)
    d2 = np.empty((B, M), np.float32)
    for b in range(B):
        r0 = results[2 * b]
        r1 = results[2 * b + 1]
        d1[b, :NSH] = -r0["d1t"].transpose(1, 0, 2).reshape(128, NT).T.reshape(-1)
        d1[b, NSH:] = -r1["d1t"].transpose(1, 0, 2).reshape(128, NT).T.reshape(-1)
        m0 = r0["d2t"].astype(np.float32).max(axis=0)
        m1 = r1["d2t"].astype(np.float32).max(axis=0)
        d2[b] = -np.maximum(m0, m1)
    return d1, d2


def kernel(xyz1, xyz2):
    xyz1 = np.asarray(xyz1, dtype=np.float32)
    xyz2 = np.asarray(xyz2, dtype=np.float32)
    in_maps = _make_in_maps(xyz1, xyz2)
    nc = _get_compiled()
    res = bass_utils.run_bass_kernel_spmd(nc, in_maps, core_ids=list(range(NCORES)))
    return _gather(res.results)
